# revision 1
# baseline (speedup 1.0000x reference)
"""BlockWiseHistogramEncoder Trainium2 kernel.

Input  x: [16, 1, 512, 512] int32, values in [0, 64).
Output:   [16, 1024, 65] float32. Image is split into 32x32 non-overlapping
16x16 blocks (row-major block order); out[b, l, 1+v] = count(v in block l)/256,
out[b, l, 0] = 0.

Sharding: pure data parallel over batch — 2 batches per core on 8 cores.

Per-core algorithm: SBUF tiles hold 128 blocks (partition = block) with the
block's 256 elements along the free dim (gathered by a strided DMA); GPSIMD
converts them to bf16. Counting is split across two engines in parallel:
  - VectorE: for low classes, tensor_scalar(is_equal, reduce-add accum_out)
    gives per-block counts at 4x 16-bit rate (one instruction per class).
  - ScalarE: for high classes, ACTIVATE(Sign, bias=-(c-0.5), accum_out) gives
    S'_c = #(v>=c) - #(v<c); adjacent differences (S'_c - S'_{c+1})/2 recover
    counts. S'_64 = -256 is a constant column.
GPSIMD does the S' differencing and the final 1/256 (1/512 for the
sign-derived columns) scaling.
"""
import sys

if "/opt/trn_rl_repo" not in sys.path:
    sys.path.insert(0, "/opt/trn_rl_repo")

import numpy as np

N_CORES = 8
B_PER_CORE = 2
H = W = 512
NC_CLS = 64
BLK = 16
HB = H // BLK          # 32 blocks per side
L = HB * HB            # 1024 blocks
E = BLK * BLK          # 256 elems per block
TILES = L // 128       # 8 tiles of 128 blocks per batch

N_ACT = 24             # classes 64-N_ACT..63 counted on ScalarE
N_DVE = NC_CLS - N_ACT # classes 0..N_DVE-1 counted on VectorE

_nc_cache = None
_run_cache = None


def _build():
    import concourse.bacc as bacc
    import concourse.mybir as mybir
    import concourse.tile as tile

    nc = bacc.Bacc("TRN2", target_bir_lowering=False, debug=False)
    x = nc.dram_tensor("x_in", [B_PER_CORE, H, W], mybir.dt.int32,
                       kind="ExternalInput")
    y = nc.dram_tensor("y_out", [B_PER_CORE, L, NC_CLS + 1], mybir.dt.float32,
                       kind="ExternalOutput")

    with tile.TileContext(nc) as tc:
        with tc.tile_pool(name="cst", bufs=1) as c_pool, \
             tc.tile_pool(name="io", bufs=6) as io_pool, \
             tc.tile_pool(name="wk", bufs=3) as w_pool, \
             tc.tile_pool(name="hs", bufs=4) as h_pool:
            # per-class ACT biases: -(c - 0.5) for c in [N_DVE, 63]
            bias = c_pool.tile([128, N_ACT], mybir.dt.float32)
            for j in range(N_ACT):
                c = N_DVE + j
                nc.vector.memset(bias[:, j:j + 1], -(c - 0.5))
            NT = B_PER_CORE * TILES
            xbs = [x.ap()[b].rearrange("(bh r) (bw c) -> bh bw r c",
                                       r=BLK, c=BLK)
                   for b in range(B_PER_CORE)]

            state = {}

            def load_stage(g):
                b, t = divmod(g, TILES)
                t_in = io_pool.tile([128, E], mybir.dt.int32)
                for i in range(4):
                    dst = t_in[32 * i:32 * (i + 1), :].rearrange(
                        "bw (r c) -> bw r c", c=BLK)
                    nc.sync.dma_start(dst, xbs[b][4 * t + i])
                t_bf = w_pool.tile([128, E], mybir.dt.bfloat16)
                nc.gpsimd.tensor_copy(t_bf[:], t_in[:])
                state[g] = t_bf

            def count_stage(g):
                b, t = divmod(g, TILES)
                t_bf = state.pop(g)
                t_h = h_pool.tile([128, NC_CLS + 1], mybir.dt.float32)
                nc.gpsimd.memset(t_h[:, 0:1], 0.0)
                # S' columns: j=0..N_ACT-1 from ACT, col N_ACT = -256
                t_s = h_pool.tile([128, N_ACT + 1], mybir.dt.float32, tag="s")
                nc.gpsimd.memset(t_s[:, N_ACT:N_ACT + 1], -256.0)
                t_tr = w_pool.tile([128, E], mybir.dt.bfloat16, tag="tr")
                t_ta = w_pool.tile([128, E], mybir.dt.bfloat16, tag="ta")
                for c in range(N_DVE):
                    nc.vector.tensor_scalar(
                        t_tr[:], t_bf[:], float(c), 0.0,
                        mybir.AluOpType.is_equal, mybir.AluOpType.add,
                        accum_out=t_h[:, c + 1:c + 2])
                for j in range(N_ACT):
                    nc.scalar.activation(
                        t_ta[:], t_bf[:],
                        mybir.ActivationFunctionType.Sign,
                        bias=bias[:, j:j + 1], scale=1.0,
                        accum_out=t_s[:, j:j + 1])
                # counts for ACT classes: (S'_c - S'_{c+1}) -> cols
                nc.gpsimd.tensor_sub(
                    t_h[:, N_DVE + 1:NC_CLS + 1],
                    t_s[:, 0:N_ACT], t_s[:, 1:N_ACT + 1])
                # final scaling: DVE cols /256, ACT cols /512
                nc.gpsimd.tensor_scalar_mul(
                    t_h[:, 1:N_DVE + 1], t_h[:, 1:N_DVE + 1], 1.0 / E)
                nc.gpsimd.tensor_scalar_mul(
                    t_h[:, N_DVE + 1:NC_CLS + 1],
                    t_h[:, N_DVE + 1:NC_CLS + 1], 1.0 / (2 * E))
                nc.sync.dma_start(y.ap()[b, 128 * t:128 * (t + 1)], t_h[:])

            # software pipeline: emit tile g+1's load/convert before tile g's
            # count/epilogue so the in-order gpsimd queue never blocks the
            # next tile's convert behind this tile's diff.
            load_stage(0)
            for g in range(NT):
                if g + 1 < NT:
                    load_stage(g + 1)
                count_stage(g)
    nc.compile()
    return nc


def _get_nc():
    global _nc_cache
    if _nc_cache is None:
        _nc_cache = _build()
    return _nc_cache


def _get_runner():
    """Build the sharded jitted executable once (run_bass_via_pjrt retraces
    per call otherwise)."""
    global _run_cache
    if _run_cache is not None:
        return _run_cache

    import jax
    from jax.sharding import Mesh, PartitionSpec
    from jax.experimental.shard_map import shard_map
    import concourse.mybir as mybir
    from concourse.bass2jax import (
        _bass_exec_p, install_neuronx_cc_hook, partition_id_tensor)

    nc = _get_nc()
    install_neuronx_cc_hook()

    partition_name = (nc.partition_id_tensor.name
                      if nc.partition_id_tensor else None)
    in_names, out_names, out_avals = [], [], []
    for alloc in nc.m.functions[0].allocations:
        if not isinstance(alloc, mybir.MemoryLocationSet):
            continue
        name = alloc.memorylocations[0].name
        if alloc.kind == "ExternalInput":
            if name != partition_name:
                in_names.append(name)
        elif alloc.kind == "ExternalOutput":
            out_names.append(name)
            out_avals.append(jax.core.ShapedArray(
                tuple(alloc.tensor_shape), mybir.dt.np(alloc.dtype)))
    n_params = len(in_names)
    n_outs = len(out_avals)
    all_in_names = list(in_names) + list(out_names)
    if partition_name is not None:
        all_in_names.append(partition_name)

    def _body(*args):
        operands = list(args)
        if partition_name is not None:
            operands.append(partition_id_tensor())
        outs = _bass_exec_p.bind(
            *operands,
            out_avals=tuple(out_avals),
            in_names=tuple(all_in_names),
            out_names=tuple(out_names),
            lowering_input_output_aliases=(),
            sim_require_finite=True,
            sim_require_nnan=True,
            nc=nc,
        )
        return tuple(outs)

    devices = jax.devices()[:N_CORES]
    mesh = Mesh(np.asarray(devices), ("core",))
    in_specs = (PartitionSpec("core"),) * (n_params + n_outs)
    out_specs = (PartitionSpec("core"),) * n_outs
    donate = tuple(range(n_params, n_params + n_outs))
    sharded = jax.jit(
        shard_map(_body, mesh=mesh, in_specs=in_specs, out_specs=out_specs,
                  check_rep=False),
        donate_argnums=donate, keep_unused=True)

    zero_shapes = [(N_CORES * a.shape[0], *a.shape[1:]) for a in out_avals]
    zero_dtypes = [a.dtype for a in out_avals]

    def run(concat_inputs):
        zeros = [np.zeros(s, d) for s, d in zip(zero_shapes, zero_dtypes)]
        out_arrs = sharded(*concat_inputs, *zeros)
        return {name: np.asarray(out_arrs[i]) for i, name in
                enumerate(out_names)}

    _run_cache = run
    return run


def kernel(x: np.ndarray) -> np.ndarray:
    assert x.shape == (16, 1, H, W) and x.dtype == np.int32, (x.shape, x.dtype)
    run = _get_runner()
    xs = np.ascontiguousarray(x[:, 0])          # [16, 512, 512] = concat of
    out = run([xs])["y_out"]                    # 8 cores' [2, 512, 512]
    return out.reshape(16, L, NC_CLS + 1).astype(np.float32, copy=False)

